# revision 27
# baseline (speedup 1.0000x reference)
"""Causal multi-head attention (B=4, T=2048, C=1024, H=16) on 8 TRN2 NeuronCores.

Sharding: core c handles batch b=c//2 and head-half r=c%2 (8 of 16 heads).
Every core runs an IDENTICAL graph (full causal attention for its 8 heads over
all T tokens) -> pure SPMD, no collectives. The output projection is
row-parallel over the head-halves; the host sums the two partial z's per batch
(the unshard step) and adds the bias-fold vector Wp@bv.

Device layout choices:
  - everything enters as bf16 (host pre-casts); matmuls accumulate fp32 in PSUM
  - all inputs are pre-arranged on the host so each DMA is fully contiguous
    per partition (max-size descriptors); loads go down two FIFO HWDGE
    queues: a critical stream (sync: first weights + front x, finely split
    so the first matmul starts ~5us in) and a bulk stream (scalar) whose
    emissions are interleaved into the schedule so they never compete with
    the critical prefix for SDMA bandwidth; z stores also ride the scalar
    ring so the sync ring stays low-latency for the normalize staging DMAs
  - qT/kT stored [d, t] with head pairs stacked 64+64 on partitions ->
    K=64 row-tiled matmul pairs use both halves of the PE array concurrently
  - scores computed transposed S^T=[k, q]; exp on ScalarE (scale=1/8 fused);
    causal handled by N-trimming each matmul + one 128x128 triangle mask mul
  - softmax denominator l = sum_k exp computed for free by an all-ones column
    appended to v (fp32 PSUM accumulation); y^T = v_aug^T @ P^T
  - softmax normalize (bf16 staging): l rows staged via DMA into [128,8] for
    one batched DVE reciprocal, then gpsimd partition_broadcast + DVE
    multiply; the tail is DEFERRED in two stages popped one per schedule
    slot, so its DMA-latency chain never head-of-line-blocks the Vector
    queue (which must keep evacuating PSUM). NOTE: the multiplies must stay
    on DVE — mixing partition_broadcast with tensor_mul on GpSimd forces a
    microcode LIBRARY_RELOAD per switch (catastrophic)
  - pair 3 walks its query blocks descending with z-projection 128-token
    groups woven in as chunk fillers so PE exp-waits are filled with z work
"""

import os
from contextlib import ExitStack

import numpy as np
import ml_dtypes

import concourse.tile as tile
from concourse import bacc, mybir


def _ensure_axon_hooks():
    """bass_utils' axon trace path does a hard import of antenv.axon_hooks,
    which this image's antenv lacks. Provide the module (with a real NTFF
    hook when the axon .so supports it) so trace=True / BASS_TRACE=1 works;
    harmless when tracing is off."""
    try:
        import antenv.axon_hooks  # noqa: F401
        return
    except ImportError:
        pass
    import sys
    import types
    try:
        import antenv
    except ImportError:
        return
    m = types.ModuleType("antenv.axon_hooks")
    m._hook = None

    def set_axon_ntff_profile_hook(h):
        m._hook = h

    def get_axon_ntff_profile_hook():
        return m._hook

    m.set_axon_ntff_profile_hook = set_axon_ntff_profile_hook
    m.get_axon_ntff_profile_hook = get_axon_ntff_profile_hook
    sys.modules["antenv.axon_hooks"] = m
    antenv.axon_hooks = m


_ensure_axon_hooks()

from concourse.bass_utils import run_bass_kernel_spmd  # noqa: E402

BF = ml_dtypes.bfloat16
B, T, C, H, HD = 4, 2048, 1024, 16, 64
NCORES = 8
DH = C // 2        # 512 d-dims per core (8 heads)
NPAIR = 4          # head pairs per core
NQB = T // 512     # 4 query blocks of 512
NKB = T // 128     # 16 key/token blocks of 128
NCH = C // 128     # 8 contraction chunks
f32 = mybir.dt.float32
bf16 = mybir.dt.bfloat16

_CACHED_NC = None
LAST_RESULTS = None  # BassKernelResults of the most recent run


def _build_nc():
    nc = bacc.Bacc("TRN2", target_bir_lowering=False, debug=False,
                   num_devices=NCORES)
    AF = mybir.ActivationFunctionType

    # host-prearranged inputs: every tensor is laid out so the DMA into its
    # SBUF tile is contiguous within each partition line
    xTh = nc.dram_tensor("xTh", [128, 4, NCH, 512], bf16,
                         kind="ExternalInput").ap()
    wqh = nc.dram_tensor("wqh", [128, NPAIR, NCH, 128], bf16,
                         kind="ExternalInput").ap()
    wkh = nc.dram_tensor("wkh", [128, NPAIR, NCH, 128], bf16,
                         kind="ExternalInput").ap()
    wvh = nc.dram_tensor("wvh", [128, NCH, DH], bf16,
                         kind="ExternalInput").ap()
    wph = nc.dram_tensor("wph", [128, NPAIR, C], bf16,
                         kind="ExternalInput").ap()
    bqkD = nc.dram_tensor("bqk", [128, 2, NPAIR, 1], f32,
                          kind="ExternalInput").ap()
    triD = nc.dram_tensor("tri", [128, 2, 128], bf16,
                          kind="ExternalInput").ap()
    zD = nc.dram_tensor("z", [T, C], mybir.dt.float16,
                    kind="ExternalOutput").ap()

    with tile.TileContext(nc) as tc, ExitStack() as ctx:
        const = ctx.enter_context(tc.tile_pool(name="const", bufs=1))
        qkp = ctx.enter_context(tc.tile_pool(name="qk", bufs=1))
        vp = ctx.enter_context(tc.tile_pool(name="vp", bufs=1))
        ynp = ctx.enter_context(tc.tile_pool(name="yn", bufs=1))
        ptp = ctx.enter_context(tc.tile_pool(name="pt", bufs=7))
        smallp = ctx.enter_context(tc.tile_pool(name="small", bufs=4))
        bcp = ctx.enter_context(tc.tile_pool(name="bc", bufs=4))
        zstp = ctx.enter_context(tc.tile_pool(name="zst", bufs=6))
        yevp = ctx.enter_context(tc.tile_pool(name="yev", bufs=8))
        stgp = ctx.enter_context(tc.tile_pool(name="stg", bufs=4))
        ps = ctx.enter_context(tc.tile_pool(name="ps", bufs=2, space="PSUM"))
        ps2 = ctx.enter_context(tc.tile_pool(name="ps2", bufs=2, space="PSUM"))
        yps = ctx.enter_context(tc.tile_pool(name="yps", bufs=2, space="PSUM"))

        xT4 = const.tile([128, 4, NCH, 512], bf16, tag="xT4", name="xT4")
        wqt = const.tile([128, NPAIR, NCH, 128], bf16, tag="wqt", name="wqt")
        wkt = const.tile([128, NPAIR, NCH, 128], bf16, tag="wkt", name="wkt")
        wvt = const.tile([128, NCH, DH], bf16, tag="wvt", name="wvt")
        wpt = const.tile([128, NPAIR, C], bf16, tag="wpt", name="wpt")
        tri2 = const.tile([128, 2, 128], bf16, tag="tri2")
        bqk = const.tile([128, 2, NPAIR, 1], f32, tag="bqk")

        # ---- critical input stream, spread across all three DMA-capable
        # queues in parallel (each queue lands its transfers serially at
        # ~2us a pop, so the critical set must not share one queue)
        nc.sync.dma_start(out=xT4[:, 0, 0:4, :], in_=xTh[:, 0, 0:4, :])
        nc.sync.dma_start(out=xT4[:, 0, 4:8, :], in_=xTh[:, 0, 4:8, :])
        nc.sync.dma_start(out=xT4[:, 1, :, :], in_=xTh[:, 1, :, :])
        nc.scalar.dma_start(out=wqt[:, 0:1, :, :], in_=wqh[:, 0:1, :, :])
        nc.scalar.dma_start(out=wkt[:, 0:1, :, :], in_=wkh[:, 0:1, :, :])
        nc.scalar.dma_start(out=bqk[:, :, :, :], in_=bqkD[:, :, :, :])
        nc.scalar.dma_start(out=wvt[:, :, :], in_=wvh[:, :, :])
        nc.gpsimd.dma_start(out=tri2[:, :, :], in_=triD[:, :, :])

        # PE warm-up: ~5us of tiny matmuls on a zeroed tile while the input
        # DMAs land, so the HAM clock gate reaches 2.4 GHz before the real
        # matmul stream begins (and stays there: the remaining DMA-wait
        # stalls are shorter than the HAM re-throttle window)
        warm = const.tile([128, 64], bf16, tag="warm")
        nc.vector.memset(warm[:, :], 0.0)
        wps = ps.tile([128, 64], f32, tag="ps", name="warmps")
        for _ in range(84):
            nc.tensor.matmul(wps[0:64, :], lhsT=warm[:, 0:64],
                             rhs=warm[:, :], start=True, stop=True)

        # bulk input stream on the scalar (HWDGE) queue; emissions are
        # interleaved into the schedule below via emit_bulk
        def bulk_loads():
            yield lambda: nc.scalar.dma_start(out=xT4[:, 2, :, :],
                                              in_=xTh[:, 2, :, :])
            yield lambda: nc.scalar.dma_start(out=xT4[:, 3, :, :],
                                              in_=xTh[:, 3, :, :])
            for hh in range(1, NPAIR):
                yield lambda hh=hh: nc.scalar.dma_start(
                    out=wqt[:, hh:hh + 1, :, :], in_=wqh[:, hh:hh + 1, :, :])
                yield lambda hh=hh: nc.scalar.dma_start(
                    out=wkt[:, hh:hh + 1, :, :], in_=wkh[:, hh:hh + 1, :, :])
            yield lambda: nc.scalar.dma_start(out=wpt[:, :, :],
                                              in_=wph[:, :, :])

        _bulk = bulk_loads()

        def emit_bulk(n=1):
            for _ in range(n):
                f = next(_bulk, None)
                if f is not None:
                    f()

        bq_sb = [bqk[:, 0, hp, :] for hp in range(NPAIR)]
        bk_sb = [bqk[:, 1, hp, :] for hp in range(NPAIR)]

        # ---- V tile: [k%128, kblock, head, 64 dims + ones column]; the
        # 65-wide weight loads cost the same as FWL 128-wide ones and need
        # no zero padding
        vt = vp.tile([128, NKB, 8, 65], bf16, tag="vt", name="vt")
        nc.vector.memset(vt[:, :, :, 64:65], 1.0)

        def emit_vproj(i0, i1):
            for i in range(i0, i1):
                p_ = ps.tile([128, DH], f32, tag="ps", name=f"vps{i}")
                for cj in range(NCH):
                    nc.tensor.matmul(p_[:, :],
                                     lhsT=xT4[:, i // 4, cj,
                                              (i % 4) * 128:(i % 4 + 1) * 128],
                                     rhs=wvt[:, cj, :],
                                     start=(cj == 0), stop=(cj == NCH - 1))
                nc.vector.tensor_copy(
                    vt[:, i, :, 0:64],
                    p_[:, :].rearrange("p (h e) -> p h e", h=8))

        yn = [ynp.tile([128, T], bf16, tag=f"yn{hp}", name=f"yn{hp}")
              for hp in range(NPAIR)]
        qts, kts = {}, {}

        def emit_qkproj_part(hp, j):
            if hp not in qts:
                qts[hp] = qkp.tile([128, T], bf16, tag=f"qT{hp}",
                                   name=f"qT{hp}")
                kts[hp] = qkp.tile([128, T], bf16, tag=f"kT{hp}",
                                   name=f"kT{hp}")
            qt, kt = qts[hp], kts[hp]
            if True:
                pq = ps.tile([128, 512], f32, tag="ps", name=f"pq{hp}_{j}")
                for cj in range(NCH):
                    nc.tensor.matmul(
                        pq[:, :],
                        lhsT=wqt[:, hp, cj, :],
                        rhs=xT4[:, j, cj, :],
                        start=(cj == 0), stop=(cj == NCH - 1))
                nc.vector.tensor_scalar_add(qt[:, j * 512:(j + 1) * 512],
                                            pq[:, :], bq_sb[hp])
                pk = ps.tile([128, 512], f32, tag="ps", name=f"pk{hp}_{j}")
                for cj in range(NCH):
                    nc.tensor.matmul(
                        pk[:, :],
                        lhsT=wkt[:, hp, cj, :],
                        rhs=xT4[:, j, cj, :],
                        start=(cj == 0), stop=(cj == NCH - 1))
                nc.vector.tensor_scalar_add(kt[:, j * 512:(j + 1) * 512],
                                            pk[:, :], bk_sb[hp])

        # deferred normalize tails: each attention block queues TWO stages
        # (1: reciprocal + rr DMA + partition broadcast; 2: the normalize
        # multiplies). One stage pops per schedule slot, so by the time a
        # stage's ops are emitted their upstream DMA/broadcast results have
        # landed and nothing head-of-line-blocks the Vector queue.
        pending_norm = []

        def slot():
            if pending_norm:
                pending_norm.pop(0)()

        def flush_norm():
            while pending_norm:
                pending_norm.pop(0)()

        def emit_attention(hp, Qi, chunk_filler=None):
                qt, kt = qts[hp], kts[hp]
                kmax = 4 * (Qi + 1)
                ya = yps.tile([128, 512], f32, tag="yps")
                yb = yps.tile([128, 512], f32, tag="yps")
                for ch in range(Qi + 1):
                    pts = []
                    for kb in range(4 * ch, 4 * ch + 4):
                        s = max(0, (kb - 4 * Qi) * 128)
                        sAB = ps2.tile([128, 2, 512], f32, tag="ps2")
                        nc.tensor.matmul(
                            sAB[:, 0, s:512],
                            lhsT=kt[0:64, kb * 128:(kb + 1) * 128],
                            rhs=qt[0:64, Qi * 512 + s:(Qi + 1) * 512],
                            start=True, stop=True)
                        nc.tensor.matmul(
                            sAB[:, 1, s:512],
                            lhsT=kt[64:128, kb * 128:(kb + 1) * 128],
                            rhs=qt[64:128, Qi * 512 + s:(Qi + 1) * 512],
                            start=True, stop=True)
                        pt_ = ptp.tile([128, 2, 512], bf16, tag="pt")
                        nc.scalar.activation(pt_[:, :, s:512],
                                             sAB[:, :, s:512],
                                             AF.Exp, scale=0.125)
                        if kb >= 4 * Qi:
                            nc.vector.tensor_mul(pt_[:, :, s:s + 128],
                                                 pt_[:, :, s:s + 128],
                                                 tri2[:, :, :])
                        pts.append((kb, s, pt_))
                    slot()
                    if chunk_filler is not None:
                        chunk_filler(ch)
                    for kb, s, pt_ in pts:
                        nc.tensor.matmul(ya[0:65, s:512],
                                         lhsT=vt[:, kb, 2 * hp, :],
                                         rhs=pt_[:, 0, s:512],
                                         start=(kb == 0), stop=(kb == kmax - 1))
                    for kb, s, pt_ in pts:
                        nc.tensor.matmul(yb[0:65, s:512],
                                         lhsT=vt[:, kb, 2 * hp + 1, :],
                                         rhs=pt_[:, 1, s:512],
                                         start=(kb == 0), stop=(kb == kmax - 1))
                # evacuate PSUM immediately (bf16 staging: halves the DVE
                # cost; ~0.4% extra error, well inside the 2e-2 budget) and
                # stage the l rows ([1,512] -> [128,4]) for one batched
                # reciprocal per block
                yevs = []
                for h, yy in ((0, ya), (1, yb)):
                    yev = yevp.tile([65, 512], bf16, tag="yev")
                    nc.vector.tensor_copy(yev[:, :], yy[0:65, :])
                    yevs.append(yev)
                stg = stgp.tile([128, 8], bf16, tag="stg")
                rstg = stgp.tile([128, 8], bf16, tag="rstg")
                for h in (0, 1):
                    nc.sync.dma_start(out=stg[:, h * 4:(h + 1) * 4],
                                      in_=yevs[h][64:65, :])
                bcs = []

                def norm_stage1():
                    with nc.allow_low_precision(
                            reason="1/l in bf16: ~0.4% rel err, well inside "
                                   "the 2e-2 budget"):
                        nc.vector.reciprocal(rstg[:, :], stg[:, :])
                    for h in (0, 1):
                        rr = smallp.tile([1, 512], bf16, tag="rr")
                        nc.sync.dma_start(out=rr[0:1, :],
                                          in_=rstg[:, h * 4:(h + 1) * 4])
                        bc = bcp.tile([64, 512], bf16, tag="bc")
                        nc.gpsimd.partition_broadcast(bc[:, :], rr[0:1, :])
                        bcs.append(bc)

                def norm_stage2():
                    for h in (0, 1):
                        nc.vector.tensor_mul(
                            yn[hp][h * 64:(h + 1) * 64,
                                   Qi * 512:(Qi + 1) * 512],
                            yevs[h][0:64, :], bcs[h])

                pending_norm.append(norm_stage1)
                pending_norm.append(norm_stage2)

        def emit_z_group(i, do_slot=True):
            # one 128-token block of the output projection: both halves of
            # wpt per yn weight-load (one LDWEIGHTS feeds two matmuls);
            # evacs on Vector, stores alternate the gpsimd/sync rings so
            # neither the ACT sequencer nor a single ring becomes the drain
            pza = ps.tile([128, 512], f32, tag="ps", name=f"pz{i}_0")
            pzb = ps.tile([128, 512], f32, tag="ps", name=f"pz{i}_1")
            for hp in range(NPAIR):
                nc.tensor.matmul(
                    pza[:, :],
                    lhsT=yn[hp][:, i * 128:(i + 1) * 128],
                    rhs=wpt[:, hp, 0:512],
                    start=(hp == 0), stop=(hp == NPAIR - 1))
                nc.tensor.matmul(
                    pzb[:, :],
                    lhsT=yn[hp][:, i * 128:(i + 1) * 128],
                    rhs=wpt[:, hp, 512:1024],
                    start=(hp == 0), stop=(hp == NPAIR - 1))
            for j2, pz in ((0, pza), (1, pzb)):
                zs = zstp.tile([128, 512], mybir.dt.float16, tag="zst")
                nc.vector.tensor_copy(zs[:, :], pz[:, :])
                eng = nc.gpsimd if (i + j2) % 2 == 0 else nc.sync
                eng.dma_start(
                    out=zD[i * 128:(i + 1) * 128,
                           j2 * 512:(j2 + 1) * 512],
                    in_=zs[:, :])
                if do_slot:
                    slot()

        def zfill(blocks):
            def f(ch):
                i = blocks.get(ch)
                if i is not None:
                    emit_z_group(i, do_slot=False)
            return f

        # ---- schedule: pair 0's attention interleaves with the
        # v-projection so ScalarE's exp stream starts early; later pairs'
        # q/k projections are spread between the previous pair's attention
        # blocks (PE filler under the ACT-bound attention stretches);
        # pair 3 walks its query blocks in descending order with the z
        # block one step behind, chased by the output projection.
        def vfill(rng):
            def f(ch):
                lo, hi = rng.get(ch, (None, None))
                if lo is not None:
                    emit_vproj(lo, hi)
            return f

        def pfill(hp_next):
            def f(ch):
                if 1 <= ch <= 3:
                    emit_qkproj_part(hp_next, ch)
            return f

        # first projection runs as quarter-chains interleaved in DMA-arrival
        # order (wq0+x[0:4] -> wk0 -> x[4:8]), so the PE never idles past the
        # HAM re-throttle window while the critical loads land serially
        qt0 = qkp.tile([128, T], bf16, tag="qT0", name="qT0")
        kt0 = qkp.tile([128, T], bf16, tag="kT0", name="kT0")
        qts[0], kts[0] = qt0, kt0
        pq0 = ps.tile([128, 512], f32, tag="ps", name="pq0_0")
        for cj in range(4):
            nc.tensor.matmul(pq0[:, :], lhsT=wqt[:, 0, cj, :],
                             rhs=xT4[:, 0, cj, :], start=(cj == 0), stop=False)
        pk0 = ps.tile([128, 512], f32, tag="ps", name="pk0_0")
        for cj in range(4):
            nc.tensor.matmul(pk0[:, :], lhsT=wkt[:, 0, cj, :],
                             rhs=xT4[:, 0, cj, :], start=(cj == 0), stop=False)
        for cj in range(4, NCH):
            nc.tensor.matmul(pq0[:, :], lhsT=wqt[:, 0, cj, :],
                             rhs=xT4[:, 0, cj, :], start=False,
                             stop=(cj == NCH - 1))
        nc.vector.tensor_scalar_add(qt0[:, 0:512], pq0[:, :], bq_sb[0])
        for cj in range(4, NCH):
            nc.tensor.matmul(pk0[:, :], lhsT=wkt[:, 0, cj, :],
                             rhs=xT4[:, 0, cj, :], start=False,
                             stop=(cj == NCH - 1))
        nc.vector.tensor_scalar_add(kt0[:, 0:512], pk0[:, :], bk_sb[0])
        emit_vproj(0, 4)
        emit_attention(0, 0)
        emit_bulk(1)                      # xT4 quarter 2
        emit_qkproj_part(0, 1)
        emit_attention(0, 1, vfill({1: (4, 8)}))
        emit_bulk(1)                      # xT4 quarter 3
        emit_qkproj_part(0, 2)
        emit_attention(0, 2, vfill({2: (8, 12)}))
        emit_bulk(2)                      # wq1, wk1
        emit_qkproj_part(0, 3)

        def fill03(ch):
            if ch < 3:
                emit_qkproj_part(1, ch)
            else:
                emit_vproj(12, 16)
                emit_qkproj_part(1, 3)

        emit_attention(0, 3, fill03)
        emit_bulk(2)                      # wq2, wk2
        for hp in (1, 2):
            for Qi in range(NQB - 1):
                emit_attention(hp, Qi)
            if hp == 1:
                emit_bulk(3)              # wq3, wk3, wpt
            emit_qkproj_part(hp + 1, 0)
            emit_attention(hp, NQB - 1, pfill(hp + 1))
        # final phase: pair-3 attention with the output projection woven in
        # as chunk fillers, so the PE's exp-waits are filled with z matmuls
        # instead of alternating big batches
        emit_attention(3, 3)
        emit_attention(3, 2, zfill({1: 12, 2: 13}))
        emit_z_group(14)
        emit_z_group(15)
        emit_attention(3, 1, zfill({1: 8}))
        emit_z_group(9)
        emit_attention(3, 0)
        for i in (10, 11, 4, 5, 6, 7):
            emit_z_group(i)
        flush_norm()
        for i in (0, 1, 2, 3):
            emit_z_group(i)

    nc.compile()
    return nc


def get_nc():
    global _CACHED_NC
    if _CACHED_NC is None:
        _CACHED_NC = _build_nc()
    return _CACHED_NC


def make_in_map(core, x, Wq, bq, Wk, bk, Wv, Wp):
    """Host-side shard/layout prep for one core (pure numpy, no FLOP-bearing
    compute: transposes, slicing, dtype casts)."""
    b, r = core // 2, core % 2
    hsl = slice(r * DH, (r + 1) * DH)
    # x[b] is [T, C]; xTh[p, qtr, cj, t] = x[b][qtr*512+t, cj*128+p]
    xTh = np.ascontiguousarray(
        x[b].reshape(4, 512, NCH, 128).transpose(3, 0, 2, 1)).astype(BF)
    # Wq[hsl] is [DH, C]; wqh[p, hp, cj, d] = Wq[hsl][hp*128+d, cj*128+p]
    wqh = np.ascontiguousarray(
        Wq[hsl].reshape(NPAIR, 128, NCH, 128).transpose(3, 0, 2, 1)).astype(BF)
    wkh = np.ascontiguousarray(
        Wk[hsl].reshape(NPAIR, 128, NCH, 128).transpose(3, 0, 2, 1)).astype(BF)
    # wvh[p, cj, d] = Wv[hsl][d, cj*128+p]
    wvh = np.ascontiguousarray(
        Wv[hsl].reshape(DH, NCH, 128).transpose(2, 1, 0)).astype(BF)
    # wph[p, hp, d] = Wp[d, hsl][hp*128+p]
    wph = np.ascontiguousarray(
        Wp[:, hsl].reshape(C, NPAIR, 128).transpose(2, 1, 0)).astype(BF)
    bqkh = np.ascontiguousarray(
        np.stack([bq[hsl].reshape(NPAIR, 128), bk[hsl].reshape(NPAIR, 128)]
                 ).transpose(2, 0, 1)[:, :, :, None]).astype(np.float32)
    tri = np.triu(np.ones((128, 128), np.float32)).astype(BF)
    tri2 = np.ascontiguousarray(np.stack([tri, tri], axis=1))
    return {
        "xTh": xTh, "wqh": wqh, "wkh": wkh, "wvh": wvh, "wph": wph,
        "bqk": bqkh, "tri": tri2,
    }


def kernel(x, Wq, bq, Wk, bk, Wv, bv, Wp):
    global LAST_RESULTS
    x = np.asarray(x, np.float32)
    Wq, bq = np.asarray(Wq, np.float32), np.asarray(bq, np.float32)
    Wk, bk = np.asarray(Wk, np.float32), np.asarray(bk, np.float32)
    Wv, bv = np.asarray(Wv, np.float32), np.asarray(bv, np.float32)
    Wp = np.asarray(Wp, np.float32)

    nc = get_nc()
    in_maps = [make_in_map(c, x, Wq, bq, Wk, bk, Wv, Wp)
               for c in range(NCORES)]
    res = None
    for attempt in range(3):
        try:
            res = run_bass_kernel_spmd(nc, in_maps,
                                       core_ids=list(range(NCORES)))
            break
        except Exception:
            if attempt == 2:
                raise
            import time
            time.sleep(5)
    LAST_RESULTS = res

    # unshard: sum the two head-half partials per batch; add folded V-bias
    # term (y gets +bv per token; through the output projection that is the
    # constant vector Wp @ bv added to every token)
    zbias = (Wp @ bv).astype(np.float32)
    out = np.empty((B, T, C), np.float32)
    for b in range(B):
        za = np.asarray(res.results[2 * b]["z"], np.float32)
        zb = np.asarray(res.results[2 * b + 1]["z"], np.float32)
        out[b] = za + zb + zbias[None, :]
    return out
